# revision 8
# baseline (speedup 1.0000x reference)
"""Block-sparse attention kernel for Trainium2 (8 NeuronCores, SPMD).

Strategy
--------
- Shard batch*heads (16 pairs) across 8 cores, 2 heads per core. The block
  mask `mat` is identical for every head, so the SPMD program (whose
  instruction stream is specialized on `mat`) is the same on all cores.
- Scores are computed transposed, S^T[k, q], so the attention matmul (AV)
  needs no transpose of the 4M-element exp'd score matrix.
- The block mask and padding mask are folded into the QK^T matmul itself by
  augmenting the contraction dimension: 32 q-block indicator rows on the Q
  side against per-(k, q-block) bias rows on the K side. Masked-off scores
  leave the PE at -30000 and exp to exactly 0 - no per-element mask work.
- QK^T uses a 2-pass bf16 hi/lo split (error ~ |q_lo|*|k_lo| ~ 1e-5, i.e.
  fp32-grade scores at bf16 speed):
    pass A (contraction 128): [q_hi; q_lo] x [k_hi; k_hi]
    pass B (contraction 96):  [q_hi; ind ] x [k_lo; bias]
- Block sparsity: k-blocks are greedily paired (host-side permutation) to
  maximize ON-column overlap within each 128-row k-tile; only the ON q-block
  runs of each k-tile are computed, packed densely in PSUM so the exp pass
  (the ACT-engine bottleneck) touches only packed columns.
- softmax denominator comes free from a ones-column appended to V (out^T row
  64 = sum of exp). Final per-q normalization happens after a cheap PE
  transpose of the [65, q] output back to [q, 65].
"""

import numpy as np
import ml_dtypes

import concourse.bass as bass
import concourse.tile as tile
from concourse import bacc, mybir
from concourse.bass_utils import run_bass_kernel_spmd
from concourse.masks import make_identity

BF16 = ml_dtypes.bfloat16
F32 = mybir.dt.float32
BF = mybir.dt.bfloat16

B, H, S, D = 2, 8, 2048, 64
BLOCK = 64
NB = S // BLOCK          # 32 blocks per side
NCORES = 8
HEADS_PER_CORE = (B * H) // NCORES   # 2
KT_ROWS = 128            # k-tile partition rows (2 k-blocks)
NKT = S // KT_ROWS       # 16 k-tiles
QHALF = 1024             # q processed in halves (PSUM budget)
NEG = -30000.0


# ----------------------------------------------------------------- planning

def _pair_kblocks(on):
    """Pair k-blocks to minimize total |union| of ON columns per pair.

    Global greedy (best overlap first) + 2-opt improvement.
    on: [NB(kb), NB(qb)] bool. Returns list of NKT (kb_i, kb_j) pairs.
    """
    ov = (on.astype(np.int32) @ on.astype(np.int32).T)
    cand = sorted(((ov[i, j], i, j) for i in range(NB)
                   for j in range(i + 1, NB)), reverse=True)
    used = [False] * NB
    pairs = []
    for o, i, j in cand:
        if not used[i] and not used[j]:
            used[i] = used[j] = True
            pairs.append((i, j))
    # 2-opt: try swapping partners between pairs
    def union(i, j):
        return int(np.sum(on[i] | on[j]))
    improved = True
    while improved:
        improved = False
        for a in range(len(pairs)):
            for b in range(a + 1, len(pairs)):
                i1, j1 = pairs[a]
                i2, j2 = pairs[b]
                cur = union(i1, j1) + union(i2, j2)
                alt1 = union(i1, i2) + union(j1, j2)
                alt2 = union(i1, j2) + union(j1, i2)
                if alt1 < cur and alt1 <= alt2:
                    pairs[a], pairs[b] = (i1, i2), (j1, j2)
                    improved = True
                elif alt2 < cur:
                    pairs[a], pairs[b] = (i1, j2), (j1, i2)
                    improved = True
    return pairs


def _plan(mat):
    """Build the sparse execution plan from the block mask.

    mat: [NB(qb), NB(kb)] int - reference masks scores[q, k] with
    mat upsampled on (axis0=q, axis1=k).
    """
    on_kb = (mat.T > 0)                      # [kb, qb]
    pairs = _pair_kblocks(on_kb)
    # element-level permutation of k
    kperm = np.concatenate(
        [np.arange(kb * BLOCK, (kb + 1) * BLOCK) for p in pairs for kb in p])
    # per k-tile ON q-blocks (union of its two k-blocks)
    on_kt = np.zeros((NKT, NB), bool)
    for t, (i, j) in enumerate(pairs):
        on_kt[t] = on_kb[i] | on_kb[j]
    first_kt = np.full(NB, NKT, np.int64)
    last_kt = np.full(NB, -1, np.int64)
    for qb in range(NB):
        kts = np.nonzero(on_kt[:, qb])[0]
        assert len(kts) > 0
        first_kt[qb], last_kt[qb] = kts[0], kts[-1]

    # per (kt, q-half): list of runs; each run is a list of sub-runs
    # (qb_start, n_qb, packed_off_cols, is_first, is_last)
    plan = []
    for t in range(NKT):
        for qh in range(S // QHALF):
            qb0, qb1 = qh * (QHALF // BLOCK), (qh + 1) * (QHALF // BLOCK)
            subruns = []
            pack = 0
            qb = qb0
            while qb < qb1:
                if not on_kt[t, qb]:
                    qb += 1
                    continue
                # maximal run of ON q-blocks, split on (first,last) changes
                flags = (first_kt[qb] == t, last_kt[qb] == t)
                start_qb = qb
                while (qb < qb1 and on_kt[t, qb]
                       and (first_kt[qb] == t, last_kt[qb] == t) == flags):
                    qb += 1
                n = qb - start_qb
                subruns.append((start_qb, n, pack, flags[0], flags[1]))
                pack += n * BLOCK
            plan.append((t, qh, subruns, pack))
    return pairs, kperm, plan


# ------------------------------------------------------------ program build

def _build_program(plan_tuple):
    pairs, kperm, plan = plan_tuple
    nc = bacc.Bacc("TRN2", target_bir_lowering=False, debug=False)

    qta = nc.dram_tensor("qta", [HEADS_PER_CORE, 128, S], mybir.dt.bfloat16,
                         kind="ExternalInput").ap()
    qtb = nc.dram_tensor("qtb", [HEADS_PER_CORE, 96, S], mybir.dt.bfloat16,
                         kind="ExternalInput").ap()
    kta = nc.dram_tensor("kta", [HEADS_PER_CORE, 128, S], mybir.dt.bfloat16,
                         kind="ExternalInput").ap()
    ktb = nc.dram_tensor("ktb", [HEADS_PER_CORE, 96, S], mybir.dt.bfloat16,
                         kind="ExternalInput").ap()
    va = nc.dram_tensor("va", [HEADS_PER_CORE, S, 65], mybir.dt.bfloat16,
                        kind="ExternalInput").ap()
    out = nc.dram_tensor("out", [HEADS_PER_CORE, S, D], F32,
                         kind="ExternalOutput").ap()

    with tile.TileContext(nc) as tc:
        with (
            tc.tile_pool(name="consts", bufs=1) as consts,
            tc.tile_pool(name="inputs", bufs=2) as inputs,
            tc.tile_pool(name="etiles", bufs=3) as etiles,
            tc.tile_pool(name="otiles", bufs=2) as otiles,
            tc.tile_pool(name="norm", bufs=4) as norm,
            tc.tile_pool(name="psS", bufs=2, space="PSUM") as psS,
            tc.tile_pool(name="psO", bufs=1, space="PSUM") as psO,
            tc.tile_pool(name="psT", bufs=2, space="PSUM") as psT,
        ):
            ident = consts.tile([128, 128], F32)
            make_identity(nc, ident)
            zeros = consts.tile([1, 577], BF)
            nc.vector.memset(zeros, 0.0)

            for h in range(HEADS_PER_CORE):
                qta_sb = inputs.tile([128, S], BF, tag="qta")
                nc.sync.dma_start(out=qta_sb, in_=qta[h])
                qtb_sb = inputs.tile([96, S], BF, tag="qtb")
                nc.sync.dma_start(out=qtb_sb, in_=qtb[h])
                kta_sb = inputs.tile([128, S], BF, tag="kta")
                nc.sync.dma_start(out=kta_sb, in_=kta[h])
                ktb_sb = inputs.tile([96, S], BF, tag="ktb")
                nc.sync.dma_start(out=ktb_sb, in_=ktb[h])
                va_sb = inputs.tile([128, NKT, 65], BF, tag="va")
                nc.sync.dma_start(
                    out=va_sb, in_=va[h].rearrange("(c p) d -> p c d", p=128))

                def chunks(off, length):
                    """Split [off, off+length) at absolute 512 (PSUM bank)
                    boundaries."""
                    o = off
                    while o < off + length:
                        w = min(off + length - o, 512 - o % 512)
                        yield o, w
                        o += w

                for qh in range(S // QHALF):
                    ot_ps = psO.tile([65, QHALF], F32, tag="ot")
                    # zero + claim has_written for each output bank via a
                    # K=1 matmul with zero weights (start=True clears the
                    # whole bank's bits; later AV matmuls pure-accumulate)
                    for bank in range(QHALF // 512):
                        nc.tensor.matmul(
                            out=ot_ps[:, bank * 512:(bank + 1) * 512],
                            lhsT=zeros[:, 0:65], rhs=zeros[:, 65:577],
                            start=True, stop=False, skip_group_check=True)
                    for t in range(NKT):
                        _, _, subruns, pack = plan[t * (S // QHALF) + qh]
                        if pack == 0:
                            continue
                        s_ps = psS.tile([128, QHALF], F32, tag="s")
                        e_sb = etiles.tile([128, QHALF], BF, tag="e")
                        # QK^T: two hi/lo passes per sub-run (bank-aligned)
                        for (qb_s, n_qb, poff, _, _) in subruns:
                            qo = qb_s * BLOCK - qh * QHALF
                            for o, w in chunks(poff, n_qb * BLOCK):
                                ro = qh * QHALF + qo + (o - poff)
                                nc.tensor.matmul(
                                    out=s_ps[:, o:o + w],
                                    lhsT=kta_sb[:, t * 128:(t + 1) * 128],
                                    rhs=qta_sb[:, ro:ro + w],
                                    start=True, stop=False)
                                nc.tensor.matmul(
                                    out=s_ps[:, o:o + w],
                                    lhsT=ktb_sb[:, t * 128:(t + 1) * 128],
                                    rhs=qtb_sb[:, ro:ro + w],
                                    start=False, stop=True)
                        nc.scalar.activation(
                            out=e_sb[:, :pack], in_=s_ps[:, :pack],
                            func=mybir.ActivationFunctionType.Exp)
                        # AV accumulate into out^T (+ ones row = sumexp)
                        for (qb_s, n_qb, poff, _, _) in subruns:
                            qo = qb_s * BLOCK - qh * QHALF
                            for o, w in chunks(qo, n_qb * BLOCK):
                                nc.tensor.matmul(
                                    out=ot_ps[:, o:o + w],
                                    lhsT=va_sb[:, t, :],
                                    rhs=e_sb[:, poff + (o - qo):
                                             poff + (o - qo) + w],
                                    start=False, stop=False,
                                    skip_group_check=True)

                    # normalize: out^T -> SBUF, PE-transpose back to [q, 65],
                    # divide by the sumexp column
                    ot_sb = otiles.tile([65, QHALF], F32, tag="otsb")
                    nc.scalar.copy(out=ot_sb, in_=ot_ps)
                    for qt in range(QHALF // 128):
                        t_ps = psT.tile([128, 65], F32, tag="t")
                        nc.tensor.transpose(
                            t_ps, ot_sb[:, qt * 128:(qt + 1) * 128],
                            ident[0:65, 0:65])
                        recip = norm.tile([128, 1], F32, tag="recip")
                        nc.vector.reciprocal(recip, t_ps[:, 64:65])
                        o_sb = norm.tile([128, D], F32, tag="o")
                        nc.vector.tensor_scalar_mul(o_sb, t_ps[:, 0:D], recip)
                        nc.sync.dma_start(
                            out=out[h, qh * QHALF + qt * 128:
                                    qh * QHALF + (qt + 1) * 128, :],
                            in_=o_sb)
    nc.compile()
    return nc


# -------------------------------------------------------------- host driver

def _split_hi_lo(x):
    hi = x.astype(BF16)
    lo = (x - hi.astype(np.float32)).astype(BF16)
    return hi, lo


def prepare(query, key, value, mask, mat):
    """Host-side prep: returns (nc, in_maps)."""
    query = np.asarray(query, np.float32)
    key = np.asarray(key, np.float32)
    value = np.asarray(value, np.float32)
    mask = np.asarray(mask, np.float32)
    mat = np.asarray(mat)

    plan_tuple = _plan(mat)
    pairs, kperm, plan = plan_tuple
    nc = _build_program(plan_tuple)

    # q-block indicator rows [NB, S]
    ind = np.zeros((NB, S), np.float32)
    for r in range(NB):
        ind[r, r * BLOCK:(r + 1) * BLOCK] = 1.0

    kb_of = np.arange(S) // BLOCK
    in_maps = []
    for c in range(NCORES):
        m = {k: np.empty(0) for k in ()}
        qta = np.empty((HEADS_PER_CORE, 128, S), BF16)
        qtb = np.empty((HEADS_PER_CORE, 96, S), BF16)
        kta = np.empty((HEADS_PER_CORE, 128, S), BF16)
        ktb = np.empty((HEADS_PER_CORE, 96, S), BF16)
        va = np.empty((HEADS_PER_CORE, S, 65), BF16)
        for i in range(HEADS_PER_CORE):
            flat = c * HEADS_PER_CORE + i
            b, h = flat // H, flat % H
            q_hi, q_lo = _split_hi_lo(query[b, h].T)        # [64, S]
            k_hi, k_lo = _split_hi_lo(key[b, h].T[:, kperm])  # permuted k
            # bias rows over permuted k: 0 (or padding bias) if block ON
            bias = np.where(
                mat[:, kb_of[kperm]] > 0,
                -1e6 * (1.0 - mask[b][kperm][None, :]), NEG
            ).astype(np.float32)
            qta[i] = np.concatenate([q_hi, q_lo], 0)
            qtb[i] = np.concatenate([q_hi, ind.astype(BF16)], 0)
            kta[i] = np.concatenate([k_hi, k_hi], 0)
            ktb[i] = np.concatenate([k_lo, bias.astype(BF16)], 0)
            va[i] = np.concatenate(
                [value[b, h][kperm], np.ones((S, 1), np.float32)],
                1).astype(BF16)
        in_maps.append({"qta": qta, "qtb": qtb, "kta": kta, "ktb": ktb,
                        "va": va})
    return nc, in_maps


def gather(results):
    out = np.empty((B, H, S, D), np.float32)
    for c in range(NCORES):
        for i in range(HEADS_PER_CORE):
            flat = c * HEADS_PER_CORE + i
            out[flat // H, flat % H] = results[c]["out"][i]
    return out


def kernel(query, key, value, mask, mat):
    nc, in_maps = prepare(query, key, value, mask, mat)
    res = run_bass_kernel_spmd(nc, in_maps, list(range(NCORES)))
    return gather(res.results)


# revision 9
# speedup vs baseline: 1783.0121x; 1783.0121x over previous
"""Block-sparse attention kernel for Trainium2 (8 NeuronCores, SPMD).

Strategy
--------
- Shard batch*heads (16 pairs) across 8 cores, 2 heads per core. The block
  mask `mat` is identical for every head, so the SPMD program (whose
  instruction stream is specialized on `mat`) is the same on all cores.
- Scores are computed transposed, S^T[k, q], so the attention matmul (AV)
  needs no transpose of the 4M-element exp'd score matrix.
- The block mask and padding mask are folded into the QK^T matmul itself by
  augmenting the contraction dimension: 32 q-block indicator rows on the Q
  side against per-(k, q-block) bias rows on the K side. Masked-off scores
  leave the PE at -30000 and exp to exactly 0 - no per-element mask work.
- QK^T runs in float32r (fp32 storage, ~11-bit-mantissa matmul at bf16
  speed for moving chunks >= 256): fp32-grade scores in a single pass.
- Block sparsity: k-blocks are greedily paired (host-side permutation) to
  maximize ON-column overlap within each 128-row k-tile, and q-blocks are
  reordered (second host-side permutation) so each k-tile's ON q-blocks
  clump into long runs. Only ON q-block runs are computed, packed densely
  in PSUM, so the exp pass (ACT engine, the 1 elem/lane/cycle bottleneck)
  touches only packed columns and matmul chunks stay long.
- softmax denominator comes free from a ones-column appended to V (out^T
  row 64 = sum of exp). Final per-q normalization happens after a cheap PE
  transpose of the [65, q] output back to [q, 65].
"""

import numpy as np
import ml_dtypes

import concourse.bass as bass
import concourse.tile as tile
from concourse import bacc, mybir
from concourse.bass_utils import run_bass_kernel_spmd
from concourse.masks import make_identity

BF16 = ml_dtypes.bfloat16
F32 = mybir.dt.float32
F32R = mybir.dt.float32r
BF = mybir.dt.bfloat16

B, H, S, D = 2, 8, 2048, 64
BLOCK = 64
NB = S // BLOCK          # 32 blocks per side
NCORES = 8
HEADS_PER_CORE = (B * H) // NCORES   # 2
KT_ROWS = 128            # k-tile partition rows (2 k-blocks)
NKT = S // KT_ROWS       # 16 k-tiles
QHALF = 1024             # q processed in halves (PSUM budget)
NEG = -30000.0


# ----------------------------------------------------------------- planning

def _pair_kblocks(on):
    """Pair k-blocks to minimize total |union| of ON columns per pair.

    Global greedy (best overlap first) + 2-opt improvement.
    on: [NB(kb), NB(qb)] bool. Returns list of NKT (kb_i, kb_j) pairs.
    """
    ov = (on.astype(np.int32) @ on.astype(np.int32).T)
    cand = sorted(((ov[i, j], i, j) for i in range(NB)
                   for j in range(i + 1, NB)), reverse=True)
    used = [False] * NB
    pairs = []
    for o, i, j in cand:
        if not used[i] and not used[j]:
            used[i] = used[j] = True
            pairs.append((i, j))

    def union(i, j):
        return int(np.sum(on[i] | on[j]))
    improved = True
    while improved:
        improved = False
        for a in range(len(pairs)):
            for b in range(a + 1, len(pairs)):
                i1, j1 = pairs[a]
                i2, j2 = pairs[b]
                cur = union(i1, j1) + union(i2, j2)
                alt1 = union(i1, i2) + union(j1, j2)
                alt2 = union(i1, j2) + union(j1, i2)
                if alt1 < cur and alt1 <= alt2:
                    pairs[a], pairs[b] = (i1, i2), (j1, j2)
                    improved = True
                elif alt2 < cur:
                    pairs[a], pairs[b] = (i1, j2), (j1, i2)
                    improved = True
    return pairs


def _order_qblocks(on_kt):
    """Order q-blocks so adjacent blocks share k-tile membership (longer
    ON runs). Greedy nearest-neighbor chain on Hamming distance."""
    cols = on_kt.T.astype(np.int8)           # [NB, NKT]
    rem = set(range(NB))
    cur = 0
    order = [cur]
    rem.discard(cur)
    while rem:
        d = {j: int(np.sum(cols[order[-1]] != cols[j])) for j in rem}
        nxt = min(d, key=lambda j: (d[j], j))
        order.append(nxt)
        rem.discard(nxt)
    # 2-opt pass on total boundary cost
    def cost(o):
        c = 0
        for a, b in zip(o[:-1], o[1:]):
            c += int(np.sum(cols[a] != cols[b]))
        return c
    best = cost(order)
    improved = True
    while improved:
        improved = False
        for i in range(1, NB - 1):
            for j in range(i + 1, NB):
                cand = order[:i] + order[i:j + 1][::-1] + order[j + 1:]
                cc = cost(cand)
                if cc < best:
                    order, best, improved = cand, cc, True
    return order


def _plan(mat):
    """Build the sparse execution plan from the block mask.

    mat: [NB(qb), NB(kb)] int - reference masks scores[q, k] with
    mat upsampled on (axis0=q, axis1=k).
    Returns (kperm, qperm, plan); plan entries are per (kt, qh) lists of
    runs (qpos_start_blocks, n_qb, packed_off_cols) in PERMUTED q space.
    """
    on_kb = (mat.T > 0)                      # [kb, qb]
    pairs = _pair_kblocks(on_kb)
    kperm = np.concatenate(
        [np.arange(kb * BLOCK, (kb + 1) * BLOCK) for p in pairs for kb in p])
    on_kt = np.zeros((NKT, NB), bool)        # [kt, qb(original)]
    for t, (i, j) in enumerate(pairs):
        on_kt[t] = on_kb[i] | on_kb[j]

    qblk_perm = _order_qblocks(on_kt)        # permuted pos -> original qb
    qperm = np.concatenate(
        [np.arange(qb * BLOCK, (qb + 1) * BLOCK) for qb in qblk_perm])
    on_kt_p = on_kt[:, qblk_perm]            # [kt, permuted qb pos]

    plan = []
    for t in range(NKT):
        for qh in range(S // QHALF):
            qb0, qb1 = qh * (QHALF // BLOCK), (qh + 1) * (QHALF // BLOCK)
            runs = []
            pack = 0
            qb = qb0
            while qb < qb1:
                if not on_kt_p[t, qb]:
                    qb += 1
                    continue
                start_qb = qb
                while qb < qb1 and on_kt_p[t, qb]:
                    qb += 1
                n = qb - start_qb
                runs.append((start_qb, n, pack))
                pack += n * BLOCK
            plan.append((t, qh, runs, pack))
    return kperm, qperm, plan


# ------------------------------------------------------------ program build

def _build_program(plan_tuple):
    kperm, qperm, plan = plan_tuple
    nc = bacc.Bacc("TRN2", target_bir_lowering=False, debug=False)

    # [q;ind] and [k;bias] fp32r tensors (contraction rows = 96)
    qtr = nc.dram_tensor("qtr", [HEADS_PER_CORE, 96, S], F32R,
                         kind="ExternalInput").ap()
    ktr = nc.dram_tensor("ktr", [HEADS_PER_CORE, 96, S], F32R,
                         kind="ExternalInput").ap()
    va = nc.dram_tensor("va", [HEADS_PER_CORE, S, 65], BF,
                        kind="ExternalInput").ap()
    out = nc.dram_tensor("out", [HEADS_PER_CORE, S, D], F32,
                         kind="ExternalOutput").ap()

    with tile.TileContext(nc) as tc:
        with (
            tc.tile_pool(name="consts", bufs=1) as consts,
            tc.tile_pool(name="inputs", bufs=2) as inputs,
            tc.tile_pool(name="etiles", bufs=3) as etiles,
            tc.tile_pool(name="otiles", bufs=2) as otiles,
            tc.tile_pool(name="norm", bufs=4) as norm,
            tc.tile_pool(name="psS", bufs=2, space="PSUM") as psS,
            tc.tile_pool(name="psO", bufs=1, space="PSUM") as psO,
            tc.tile_pool(name="psT", bufs=2, space="PSUM") as psT,
        ):
            ident = consts.tile([128, 128], F32)
            make_identity(nc, ident)
            zeros = consts.tile([1, 577], BF)
            nc.vector.memset(zeros, 0.0)

            def chunks(off, length):
                o = off
                while o < off + length:
                    w = min(off + length - o, 512 - o % 512)
                    yield o, w
                    o += w

            for h in range(HEADS_PER_CORE):
                qtr_sb = inputs.tile([96, S], F32R, tag="qtr")
                nc.sync.dma_start(out=qtr_sb, in_=qtr[h])
                ktr_sb = inputs.tile([96, S], F32R, tag="ktr")
                nc.sync.dma_start(out=ktr_sb, in_=ktr[h])
                va_sb = inputs.tile([128, NKT, 65], BF, tag="va")
                nc.sync.dma_start(
                    out=va_sb, in_=va[h].rearrange("(c p) d -> p c d", p=128))

                for qh in range(S // QHALF):
                    ot_ps = psO.tile([65, QHALF], F32, tag="ot")
                    # zero + claim has_written for each output bank via a
                    # K=1 matmul with zero weights; AV then pure-accumulates
                    for bank in range(QHALF // 512):
                        nc.tensor.matmul(
                            out=ot_ps[:, bank * 512:(bank + 1) * 512],
                            lhsT=zeros[:, 0:65], rhs=zeros[:, 65:577],
                            start=True, stop=False, skip_group_check=True)
                    for t in range(NKT):
                        _, _, runs, pack = plan[t * (S // QHALF) + qh]
                        if pack == 0:
                            continue
                        s_ps = psS.tile([128, QHALF], F32, tag="s")
                        e_sb = etiles.tile([128, QHALF], BF, tag="e")
                        # QK^T single fp32r pass per bank-aligned chunk
                        for (qb_s, n_qb, poff) in runs:
                            qo = qb_s * BLOCK - qh * QHALF
                            for o, w in chunks(poff, n_qb * BLOCK):
                                ro = qh * QHALF + qo + (o - poff)
                                nc.tensor.matmul(
                                    out=s_ps[:, o:o + w],
                                    lhsT=ktr_sb[:, t * 128:(t + 1) * 128],
                                    rhs=qtr_sb[:, ro:ro + w],
                                    start=True, stop=True)
                        nc.scalar.activation(
                            out=e_sb[:, :pack], in_=s_ps[:, :pack],
                            func=mybir.ActivationFunctionType.Exp)
                        # AV accumulate into out^T (+ ones row = sumexp)
                        for (qb_s, n_qb, poff) in runs:
                            qo = qb_s * BLOCK - qh * QHALF
                            for o, w in chunks(qo, n_qb * BLOCK):
                                nc.tensor.matmul(
                                    out=ot_ps[:, o:o + w],
                                    lhsT=va_sb[:, t, :],
                                    rhs=e_sb[:, poff + (o - qo):
                                             poff + (o - qo) + w],
                                    start=False, stop=False,
                                    skip_group_check=True)

                    # normalize: out^T -> SBUF, PE-transpose back to [q, 65],
                    # divide by the sumexp column
                    ot_sb = otiles.tile([65, QHALF], F32, tag="otsb")
                    nc.vector.tensor_copy(ot_sb, ot_ps)
                    for qt in range(QHALF // 128):
                        t_ps = psT.tile([128, 65], F32, tag="t")
                        nc.tensor.transpose(
                            t_ps, ot_sb[:, qt * 128:(qt + 1) * 128],
                            ident[0:65, 0:65])
                        recip = norm.tile([128, 1], F32, tag="recip")
                        nc.vector.reciprocal(recip, t_ps[:, 64:65])
                        o_sb = norm.tile([128, D], F32, tag="o")
                        nc.vector.tensor_scalar_mul(o_sb, t_ps[:, 0:D], recip)
                        nc.sync.dma_start(
                            out=out[h, qh * QHALF + qt * 128:
                                    qh * QHALF + (qt + 1) * 128, :],
                            in_=o_sb)
    nc.compile()
    return nc


# -------------------------------------------------------------- host driver

def prepare(query, key, value, mask, mat):
    """Host-side prep: returns (nc, in_maps, qperm)."""
    query = np.asarray(query, np.float32)
    key = np.asarray(key, np.float32)
    value = np.asarray(value, np.float32)
    mask = np.asarray(mask, np.float32)
    mat = np.asarray(mat)

    plan_tuple = _plan(mat)
    kperm, qperm, plan = plan_tuple
    nc = _build_program(plan_tuple)

    # q-block indicator rows in PERMUTED q space: row r marks positions
    # whose ORIGINAL q-block is r
    ind = (np.arange(S)[None, :] // BLOCK == 0)  # placeholder
    orig_qb = qperm // BLOCK                     # [S] permuted pos -> orig qb
    ind = (orig_qb[None, :] == np.arange(NB)[:, None]).astype(np.float32)

    kb_of = np.arange(S) // BLOCK
    in_maps = []
    for c in range(NCORES):
        qtr = np.empty((HEADS_PER_CORE, 96, S), np.float32)
        ktr = np.empty((HEADS_PER_CORE, 96, S), np.float32)
        va = np.empty((HEADS_PER_CORE, S, 65), BF16)
        for i in range(HEADS_PER_CORE):
            flat = c * HEADS_PER_CORE + i
            b, h = flat // H, flat % H
            # bias rows over permuted k: row r (orig qb r) vs k-block
            bias = np.where(
                mat[:, kb_of[kperm]] > 0,
                -1e6 * (1.0 - mask[b][kperm][None, :]), NEG
            ).astype(np.float32)
            qtr[i] = np.concatenate([query[b, h].T[:, qperm], ind], 0)
            ktr[i] = np.concatenate([key[b, h].T[:, kperm], bias], 0)
            va[i] = np.concatenate(
                [value[b, h][kperm], np.ones((S, 1), np.float32)],
                1).astype(BF16)
        in_maps.append({"qtr": qtr, "ktr": ktr, "va": va})
    return nc, in_maps, qperm


def gather(results, qperm):
    out = np.empty((B, H, S, D), np.float32)
    for c in range(NCORES):
        for i in range(HEADS_PER_CORE):
            flat = c * HEADS_PER_CORE + i
            out[flat // H, flat % H][qperm] = results[c]["out"][i]
    return out


def kernel(query, key, value, mask, mat):
    nc, in_maps, qperm = prepare(query, key, value, mask, mat)
    res = run_bass_kernel_spmd(nc, in_maps, list(range(NCORES)))
    return gather(res.results, qperm)


# revision 25
# speedup vs baseline: 2410.3633x; 1.3518x over previous
"""Block-sparse attention kernel for Trainium2 (8 NeuronCores, SPMD).

Strategy
--------
- Shard batch*heads (16 pairs) across 8 cores, 2 heads per core. The block
  mask `mat` is identical for every head, so the SPMD program (whose
  instruction stream is specialized on `mat`) is the same on all cores.
- Scores are computed transposed, S^T[k, q], so the attention matmul (AV)
  needs no transpose of the 4M-element exp'd score matrix.
- The block mask and padding mask are folded into the QK^T matmul itself by
  augmenting the contraction dimension: 32 q-block indicator rows on the Q
  side against per-(k, q-block) bias rows on the K side. Masked-off scores
  leave the PE at -30000 and exp to exactly 0 - no per-element mask work.
- QK^T runs in float32r (fp32 storage, ~11-bit-mantissa matmul at bf16
  speed for moving chunks >= 256): fp32-grade scores in a single pass.
- Block sparsity: k-blocks are greedily paired (host-side permutation) to
  maximize ON-column overlap within each 128-row k-tile, and q-blocks are
  reordered (second host-side permutation) so each k-tile's ON q-blocks
  clump into long runs. Only ON q-block runs are computed, packed densely
  in PSUM, so the exp pass (ACT engine, the 1 elem/lane/cycle bottleneck)
  touches only packed columns and matmul chunks stay long.
- softmax denominator comes free from a ones-column appended to V (out^T
  row 64 = sum of exp). The raw [65, q] out^T accumulator is shipped to the
  host, which does the final transpose and divide (free relative to the
  device pipeline).
- Engines are software-pipelined by emission order: AV(t-1) after QK(t),
  each q-half's output store deferred past the next half's leading QK
  tiles, so the PE never sits behind the exp it depends on.
"""

import numpy as np
import ml_dtypes

import concourse.bass as bass
import concourse.tile as tile
from concourse import bacc, mybir
from concourse.bass_utils import run_bass_kernel_spmd
BF16 = ml_dtypes.bfloat16
F32 = mybir.dt.float32
F32R = mybir.dt.float32r
BF = mybir.dt.bfloat16

B, H, S, D = 2, 8, 2048, 64
BLOCK = 64
NB = S // BLOCK          # 32 blocks per side
NCORES = 8
HEADS_PER_CORE = (B * H) // NCORES   # 2
KT_ROWS = 128            # k-tile partition rows (2 k-blocks)
NKT = S // KT_ROWS       # 16 k-tiles
QHALF = 1024             # q processed in halves (PSUM budget)
NEG = -30000.0


# ----------------------------------------------------------------- planning

def _pair_kblocks(on):
    """Pair k-blocks to minimize total |union| of ON columns per pair.

    Global greedy (best overlap first) + 2-opt improvement.
    on: [NB(kb), NB(qb)] bool. Returns list of NKT (kb_i, kb_j) pairs.
    """
    ov = (on.astype(np.int32) @ on.astype(np.int32).T)
    cand = sorted(((ov[i, j], i, j) for i in range(NB)
                   for j in range(i + 1, NB)), reverse=True)
    used = [False] * NB
    pairs = []
    for o, i, j in cand:
        if not used[i] and not used[j]:
            used[i] = used[j] = True
            pairs.append((i, j))

    def union(i, j):
        return int(np.sum(on[i] | on[j]))
    improved = True
    while improved:
        improved = False
        for a in range(len(pairs)):
            for b in range(a + 1, len(pairs)):
                i1, j1 = pairs[a]
                i2, j2 = pairs[b]
                cur = union(i1, j1) + union(i2, j2)
                alt1 = union(i1, i2) + union(j1, j2)
                alt2 = union(i1, j2) + union(j1, i2)
                if alt1 < cur and alt1 <= alt2:
                    pairs[a], pairs[b] = (i1, i2), (j1, j2)
                    improved = True
                elif alt2 < cur:
                    pairs[a], pairs[b] = (i1, j2), (j1, i2)
                    improved = True
    return pairs


def _order_qblocks(on_kt):
    """Order q-blocks so adjacent blocks share k-tile membership (longer
    ON runs). Greedy nearest-neighbor chain on Hamming distance."""
    cols = on_kt.T.astype(np.int8)           # [NB, NKT]
    rem = set(range(NB))
    cur = 0
    order = [cur]
    rem.discard(cur)
    while rem:
        d = {j: int(np.sum(cols[order[-1]] != cols[j])) for j in rem}
        nxt = min(d, key=lambda j: (d[j], j))
        order.append(nxt)
        rem.discard(nxt)
    # 2-opt pass on total boundary cost
    def cost(o):
        c = 0
        for a, b in zip(o[:-1], o[1:]):
            c += int(np.sum(cols[a] != cols[b]))
        return c
    best = cost(order)
    improved = True
    while improved:
        improved = False
        for i in range(1, NB - 1):
            for j in range(i + 1, NB):
                cand = order[:i] + order[i:j + 1][::-1] + order[j + 1:]
                cc = cost(cand)
                if cc < best:
                    order, best, improved = cand, cc, True
    return order


def _plan(mat):
    """Build the sparse execution plan from the block mask.

    mat: [NB(qb), NB(kb)] int - reference masks scores[q, k] with
    mat upsampled on (axis0=q, axis1=k).
    Returns (kperm, qperm, plan); plan entries are per (kt, qh) lists of
    runs (qpos_start_blocks, n_qb, packed_off_cols) in PERMUTED q space.
    """
    on_kb = (mat.T > 0)                      # [kb, qb]
    pairs = _pair_kblocks(on_kb)
    kperm = np.concatenate(
        [np.arange(kb * BLOCK, (kb + 1) * BLOCK) for p in pairs for kb in p])
    on_kt = np.zeros((NKT, NB), bool)        # [kt, qb(original)]
    for t, (i, j) in enumerate(pairs):
        on_kt[t] = on_kb[i] | on_kb[j]

    qblk_perm = _order_qblocks(on_kt)        # permuted pos -> original qb
    qperm = np.concatenate(
        [np.arange(qb * BLOCK, (qb + 1) * BLOCK) for qb in qblk_perm])
    on_kt_p = on_kt[:, qblk_perm]            # [kt, permuted qb pos]

    plan = []
    for t in range(NKT):
        for qh in range(S // QHALF):
            qb0, qb1 = qh * (QHALF // BLOCK), (qh + 1) * (QHALF // BLOCK)
            runs = []
            pack = 0
            qb = qb0
            while qb < qb1:
                if not on_kt_p[t, qb]:
                    qb += 1
                    continue
                start_qb = qb
                while qb < qb1 and on_kt_p[t, qb]:
                    qb += 1
                n = qb - start_qb
                runs.append((start_qb, n, pack))
                pack += n * BLOCK
            plan.append((t, qh, runs, pack))
    return kperm, qperm, plan


# ------------------------------------------------------------ program build

def _build_program(plan_tuple):
    kperm, qperm, plan = plan_tuple
    nc = bacc.Bacc("TRN2", target_bir_lowering=False, debug=False)

    # [q;ind] and [k;bias] fp32r tensors (contraction rows = 96)
    qtr = nc.dram_tensor("qtr", [HEADS_PER_CORE, 96, S], F32R,
                         kind="ExternalInput").ap()
    ktr = nc.dram_tensor("ktr", [HEADS_PER_CORE, 96, S], F32R,
                         kind="ExternalInput").ap()
    va = nc.dram_tensor("va", [HEADS_PER_CORE, S, 65], BF,
                        kind="ExternalInput").ap()
    # raw out^T accumulator halves [65, QHALF]; host transposes and
    # divides by the sumexp row (row 64)
    out = nc.dram_tensor("out", [HEADS_PER_CORE, S // QHALF, 65, QHALF],
                         F32, kind="ExternalOutput").ap()

    with tile.TileContext(nc) as tc:
        with (
            tc.tile_pool(name="consts", bufs=1) as consts,
            tc.tile_pool(name="inputs", bufs=2) as inputs,
            tc.tile_pool(name="etiles", bufs=3) as etiles,
            tc.tile_pool(name="otiles", bufs=2) as otiles,
            tc.tile_pool(name="psS", bufs=2, space="PSUM") as psS,
            tc.tile_pool(name="psO", bufs=2, space="PSUM") as psO,
        ):
            zeros = consts.tile([1, 577], BF)
            nc.vector.memset(zeros, 0.0)

            def chunks(off, length):
                o = off
                while o < off + length:
                    w = min(off + length - o, 512 - o % 512)
                    yield o, w
                    o += w

            # warm the exp table during the initial input DMAs
            warm = consts.tile([1, 1], F32)
            nc.scalar.activation(out=warm, in_=zeros[:, 0:1].bitcast(BF),
                                 func=mybir.ActivationFunctionType.Exp)

            def emit_qk(sbufs, qh, t, runs, pack):
                qtr_sb, ktr_sb, va_sb = sbufs
                s_ps = psS.tile([128, QHALF], F32, tag="s")
                e_sb = etiles.tile([128, QHALF], BF, tag="e")
                # QK^T, single fp32r pass per bank-aligned chunk.
                # Sub-256 chunks run 4x slower (fp32r moving-rate), so pad
                # them with throwaway columns up to 256 when the PSUM bank
                # and the rhs tensor have room.
                for (qb_s, n_qb, poff) in runs:
                    qo = qb_s * BLOCK - qh * QHALF
                    for o, w in chunks(poff, n_qb * BLOCK):
                        ro = qh * QHALF + qo + (o - poff)
                        wp = w
                        if w < 256:
                            room = min(512 - o % 512, QHALF - o, S - ro)
                            wp = max(min(256, room), w)
                        nc.tensor.matmul(
                            out=s_ps[:, o:o + wp],
                            lhsT=ktr_sb[:, t * 128:(t + 1) * 128],
                            rhs=qtr_sb[:, ro:ro + wp],
                            start=True, stop=True)
                nc.scalar.activation(
                    out=e_sb[:, :pack], in_=s_ps[:, :pack],
                    func=mybir.ActivationFunctionType.Exp)
                return e_sb

            def emit_av(sbufs, ot_ps, qh, t, runs, e_sb):
                va_sb = sbufs[2]
                # AV accumulate into out^T (+ ones row = sumexp)
                for (qb_s, n_qb, poff) in runs:
                    qo = qb_s * BLOCK - qh * QHALF
                    for o, w in chunks(qo, n_qb * BLOCK):
                        nc.tensor.matmul(
                            out=ot_ps[:, o:o + w],
                            lhsT=va_sb[:, t, :],
                            rhs=e_sb[:, poff + (o - qo):
                                     poff + (o - qo) + w],
                            start=False, stop=False,
                            skip_group_check=True)

            def emit_norm(h, qh, ot_ps):
                # ship the raw out^T accumulator; host normalizes.
                # PSUM -> SBUF copy split across DVE and ACT, then DMA.
                ot_sb = otiles.tile([65, QHALF], F32, tag="otsb")
                nc.vector.tensor_copy(ot_sb[:, :QHALF // 2],
                                      ot_ps[:, :QHALF // 2])
                nc.scalar.copy(ot_sb[:, QHALF // 2:], ot_ps[:, QHALF // 2:])
                nc.sync.dma_start(out=out[h, qh], in_=ot_sb)

            sbufs = None
            deferred_norm = None
            for h in range(HEADS_PER_CORE):
                # spread big input loads across distinct DMA queues; ktr
                # is chunked so the first k-tiles land early
                qtr_sb = inputs.tile([96, S], F32R, tag="qtr")
                for qc in range(2):
                    nc.sync.dma_start(
                        out=qtr_sb[:, qc * QHALF:(qc + 1) * QHALF],
                        in_=qtr[h][:, qc * QHALF:(qc + 1) * QHALF])
                ktr_sb = inputs.tile([96, S], F32R, tag="ktr")
                for kc in range(4):
                    nc.scalar.dma_start(
                        out=ktr_sb[:, kc * 512:(kc + 1) * 512],
                        in_=ktr[h][:, kc * 512:(kc + 1) * 512])
                va_sb = inputs.tile([128, NKT, 65], BF, tag="va")
                nc.gpsimd.dma_start(
                    out=va_sb, in_=va[h].rearrange("(c p) d -> p c d", p=128))
                sbufs = (qtr_sb, ktr_sb, va_sb)

                for qh in range(S // QHALF):
                    tiles_q = [plan[t * (S // QHALF) + qh] for t in range(NKT)]
                    tiles_q = [(t, runs, pack) for (t, _, runs, pack)
                               in tiles_q if pack > 0]

                    # two leading QK tiles, then the previous q-half's
                    # normalize + this half's PSUM zeroing, then the steady
                    # QK(t)/AV(t-1) pipeline
                    lead = tiles_q[:2]
                    eq = []
                    for (t, runs, pack) in lead:
                        eq.append((t, runs, emit_qk(sbufs, qh, t, runs,
                                                    pack)))
                    if deferred_norm is not None:
                        deferred_norm()
                        deferred_norm = None
                    ot_ps = psO.tile([65, QHALF], F32, tag="ot")
                    # zero + claim has_written for each output bank via a
                    # K=1 matmul with zero weights; AV pure-accumulates
                    for bank in range(QHALF // 512):
                        nc.tensor.matmul(
                            out=ot_ps[:, bank * 512:(bank + 1) * 512],
                            lhsT=zeros[:, 0:65], rhs=zeros[:, 65:577],
                            start=True, stop=False, skip_group_check=True)
                    pending = list(eq)
                    for (t, runs, pack) in tiles_q[2:]:
                        e_sb = emit_qk(sbufs, qh, t, runs, pack)
                        if pending:
                            pt, pruns, pe = pending.pop(0)
                            emit_av(sbufs, ot_ps, qh, pt, pruns, pe)
                        pending.append((t, runs, e_sb))
                    for (pt, pruns, pe) in pending:
                        emit_av(sbufs, ot_ps, qh, pt, pruns, pe)

                    deferred_norm = (lambda hh=h, qq=qh, op=ot_ps:
                                     emit_norm(hh, qq, op))
            if deferred_norm is not None:
                deferred_norm()
    nc.compile()
    return nc


# -------------------------------------------------------------- host driver

def prepare(query, key, value, mask, mat):
    """Host-side prep: returns (nc, in_maps, qperm)."""
    query = np.asarray(query, np.float32)
    key = np.asarray(key, np.float32)
    value = np.asarray(value, np.float32)
    mask = np.asarray(mask, np.float32)
    mat = np.asarray(mat)

    plan_tuple = _plan(mat)
    kperm, qperm, plan = plan_tuple
    nc = _build_program(plan_tuple)

    # q-block indicator rows in PERMUTED q space: row r marks positions
    # whose ORIGINAL q-block is r
    ind = (np.arange(S)[None, :] // BLOCK == 0)  # placeholder
    orig_qb = qperm // BLOCK                     # [S] permuted pos -> orig qb
    ind = (orig_qb[None, :] == np.arange(NB)[:, None]).astype(np.float32)

    kb_of = np.arange(S) // BLOCK

    in_maps = []
    for c in range(NCORES):
        qtr = np.empty((HEADS_PER_CORE, 96, S), np.float32)
        ktr = np.empty((HEADS_PER_CORE, 96, S), np.float32)
        va = np.empty((HEADS_PER_CORE, S, 65), BF16)
        for i in range(HEADS_PER_CORE):
            flat = c * HEADS_PER_CORE + i
            b, h = flat // H, flat % H
            # bias rows over permuted k: row r (orig qb r) vs k-block
            bias = np.where(
                mat[:, kb_of[kperm]] > 0,
                -1e6 * (1.0 - mask[b][kperm][None, :]), NEG
            ).astype(np.float32)
            qtr[i] = np.concatenate([query[b, h].T[:, qperm], ind], 0)
            ktr[i] = np.concatenate([key[b, h].T[:, kperm], bias], 0)
            va[i] = np.concatenate(
                [value[b, h][kperm], np.ones((S, 1), np.float32)],
                1).astype(BF16)
        in_maps.append({"qtr": qtr, "ktr": ktr, "va": va})
    return nc, in_maps, qperm


def gather(results, qperm):
    out = np.empty((B, H, S, D), np.float32)
    for c in range(NCORES):
        for i in range(HEADS_PER_CORE):
            flat = c * HEADS_PER_CORE + i
            oth = results[c]["out"][i]          # [S//QHALF, 65, QHALF]
            ot = np.concatenate([oth[j] for j in range(S // QHALF)], 1)
            out[flat // H, flat % H][qperm] = (ot[:D] / ot[D:D + 1]).T
    return out


def kernel(query, key, value, mask, mat):
    nc, in_maps, qperm = prepare(query, key, value, mask, mat)
    res = run_bass_kernel_spmd(nc, in_maps, list(range(NCORES)))
    return gather(res.results, qperm)
